# revision 3
# baseline (speedup 1.0000x reference)
"""Causal self-attention (GQA, RoPE) on 8 Trainium2 NeuronCores.

Sharding: tensor-parallel by KV-head group. Core c owns kv-head c and its 4
query heads, for both batch elements. Each core computes
  qkv^T slice -> RoPE -> causal attention -> out-projection partial
and the host sums the 8 partial out-projection results (Wout row-sharded),
which replaces the all-reduce.

All matmul operands are bf16 (PE runs bf16 at full rate; accumulation is
fp32 in PSUM). Intermediates stay SBUF-resident across phases: the roped
qkv^T blocks and the attention outputs never round-trip through DRAM. RoPE
is fused into the qkv PSUM->SBUF evacuation on the vector engine.

Layouts (per core, s = b*S + pos, SQ = B*S):
  xT    [H, SQ]  bf16   x transposed
  w3    [H, 768] bf16   [Wq(4 heads, pre-scaled by 1/sqrt(hd)) | Wk | Wv]
  wout  [512, H] bf16   Wout rows for this core's 4 q heads
  cosT  [128, SQ] bf16  cos table transposed, tiled over batches
  sinS  [128, SQ] bf16  sin table, rows 0:64 negated (rotate_half baked in)
  maskT [128, 4*512] bf16  causal 0/1 mask for the 4 diagonal k-chunks
Output: outT [H, SQ] fp16 (partial out-projection, transposed; host sums in
fp32).
"""
import numpy as np

import concourse.bass as bass
import concourse.mybir as mybir
import concourse.tile as tile
from concourse import bacc
from concourse.masks import make_identity

F32 = mybir.dt.float32
BF = mybir.dt.bfloat16
F16 = mybir.dt.float16
P = 128

N_CORES = 8
CFG = dict(B=2, S=2048, H=4096, HD=128, NQ=4)  # NQ = q heads per core


def build(cfg=CFG, reps=1, phases=(1, 2, 3)):
    B, S, H, HD, NQ = cfg["B"], cfg["S"], cfg["H"], cfg["HD"], cfg["NQ"]
    SQ = B * S
    HCH = H // P          # h chunks (32)
    C6 = NQ + 2           # c-tiles: NQ q heads, 1 k, 1 v
    CW = C6 * P           # qkv out width per core (768)
    NSB = SQ // 512       # 512-wide s blocks (8)
    QB = S // 512         # q blocks per batch (4)
    SCH = S // P          # k chunks per batch (16)
    h2 = HD // 2

    nc = bacc.Bacc("TRN2", target_bir_lowering=False, debug=False,
                   num_devices=N_CORES)
    xT = nc.dram_tensor("xT", [H, SQ], BF, kind="ExternalInput").ap()
    w3 = nc.dram_tensor("w3", [H, CW], BF, kind="ExternalInput").ap()
    wout = nc.dram_tensor("wout", [NQ * P, H], BF, kind="ExternalInput").ap()
    cosT = nc.dram_tensor("cosT", [P, SQ], BF, kind="ExternalInput").ap()
    sinS = nc.dram_tensor("sinS", [P, SQ], BF, kind="ExternalInput").ap()
    maskT = nc.dram_tensor("maskT", [P, 4 * 512], BF, kind="ExternalInput").ap()
    outT = nc.dram_tensor("outT", [H, SQ], F16, kind="ExternalOutput").ap()

    xT_v = xT.rearrange("(ho p) s -> p ho s", p=P)      # [128, HCH, SQ]
    w3_v = w3.rearrange("(ho p) c -> p ho c", p=P)      # [128, HCH, CW]
    wout_v = wout.rearrange("(co p) n -> p co n", p=P)  # [128, NQ, H]
    outT_v = outT.rearrange("(ho p) s -> p ho s", p=P)  # [128, HCH, SQ]

    with tile.TileContext(nc, pool_alloc_mode="queue") as tc:
        def body(iv=None):
            with tc.tile_pool(name="pers", bufs=1) as pers, \
                 tc.tile_pool(name="pqkv", bufs=1) as pqkv:
                # persists through phase 3
                att_sb = [pers.tile([P, NQ, 512], BF, name=f"att{i}")
                          for i in range(NSB)]
                ones_b = pers.tile([P, P], BF)
                ident_b = pers.tile([P, P], BF)
                # persists through phase 2
                qkv_sb = [pqkv.tile([P, C6, 512], BF, name=f"qkv{i}")
                          for i in range(NSB)]

                # ------------- Phase 1: qkv^T = w3^T @ x^T, rope fused -------------
                if 1 in phases:
                  with tc.tile_pool(name="p1w", bufs=1) as p1w, \
                     tc.tile_pool(name="p1x", bufs=3) as p1x, \
                     tc.tile_pool(name="p1cs", bufs=2) as p1cs, \
                     tc.tile_pool(name="p1t", bufs=2) as p1t, \
                     tc.tile_pool(name="ps1", bufs=6, space="PSUM") as ps1:
                    ones_f = p1w.tile([P, P], F32)
                    nc.vector.memset(ones_f[:], 1.0)
                    nc.vector.tensor_copy(ones_b[:], ones_f[:])
                    ident_f = p1w.tile([P, P], F32)
                    make_identity(nc, ident_f[:])
                    nc.vector.tensor_copy(ident_b[:], ident_f[:])

                    w3_t = p1w.tile([P, HCH, CW], BF)
                    for g in range(2):
                        nc.sync.dma_start(w3_t[:, g * 16:(g + 1) * 16, :],
                                          w3_v[:, g * 16:(g + 1) * 16, :])

                    for sb in range(NSB):
                        soff = sb * 512
                        xt = []
                        for g in range(2):
                            xh = p1x.tile([P, 16, 512], BF, name="xt", tag="xt")
                            nc.sync.dma_start(
                                xh[:], xT_v[:, g * 16:(g + 1) * 16, soff:soff + 512])
                            xt.append(xh)
                        cs = p1cs.tile([P, 512], BF, name="cs", tag="cs")
                        sn = p1cs.tile([P, 512], BF, name="sn", tag="sn")
                        nc.sync.dma_start(cs[:], cosT[:, soff:soff + 512])
                        nc.sync.dma_start(sn[:], sinS[:, soff:soff + 512])
                        for ci in range(C6):
                            ps = ps1.tile([P, 512], F32, name="p1p", tag="p1p")
                            for hc in range(HCH):
                                nc.tensor.matmul(
                                    ps[:],
                                    w3_t[:, hc, ci * P:(ci + 1) * P],
                                    xt[hc // 16][:, hc % 16, :],
                                    start=(hc == 0), stop=(hc == HCH - 1),
                                )
                            dst = qkv_sb[sb][:, ci, :]
                            if ci < NQ + 1:
                                # rope: dst = ps*cos + rot_half(ps)*sinS
                                t1 = p1t.tile([P, 512], BF, name="t1", tag="t1")
                                t2 = p1t.tile([P, 512], BF, name="t2", tag="t2")
                                nc.vector.tensor_mul(t1[:], ps[:], cs[:])
                                nc.vector.tensor_mul(
                                    t2[:h2, :], ps[h2:2 * h2, :], sn[:h2, :])
                                nc.vector.tensor_mul(
                                    t2[h2:2 * h2, :], ps[:h2, :], sn[h2:2 * h2, :])
                                nc.vector.tensor_add(dst, t1[:], t2[:])
                            else:
                                nc.vector.tensor_copy(dst, ps[:])

                # ------------- Phase 2: attention -------------
                if 2 in phases:
                  with tc.tile_pool(name="p2c", bufs=1) as p2c, \
                     tc.tile_pool(name="p2pt", bufs=2) as p2pt, \
                     tc.tile_pool(name="p2w", bufs=2) as p2w, \
                     tc.tile_pool(name="ps2", bufs=1, space="PSUM") as ps2:
                    mask_t = p2c.tile([P, 4, 512], BF)
                    nc.sync.dma_start(
                        mask_t[:], maskT.rearrange("p (v q) -> p v q", v=4))
                    v_rb = [p2c.tile([P, SCH, HD], BF, name=f"vr{b}")
                            for b in range(B)]

                    def attn_scores(b, qb, h, pT):
                        nch = (qb + 1) * 4
                        q_ap = qkv_sb[b * QB + qb][:, h, :]
                        for kc in range(nch):
                            k_ap = qkv_sb[b * QB + kc // 4][
                                :, NQ, (kc % 4) * P:(kc % 4 + 1) * P]
                            sps = ps2.tile([P, 512], F32, name="sc", tag="sc",
                                           bufs=3)
                            nc.tensor.matmul(sps[:], k_ap, q_ap,
                                             start=True, stop=True)
                            nc.scalar.activation(
                                pT[:, kc, :], sps[:],
                                mybir.ActivationFunctionType.Exp)
                            if kc >= nch - 4:
                                nc.gpsimd.tensor_mul(
                                    pT[:, kc, :], pT[:, kc, :],
                                    mask_t[:, kc - (nch - 4), :])

                    def attn_av(b, qb, h, pT):
                        nch = (qb + 1) * 4
                        lps = ps2.tile([P, 512], F32, name="lp", tag="lp",
                                       bufs=2)
                        ops = ps2.tile([P, 512], F32, name="av", tag="av",
                                       bufs=2)
                        for kc in range(nch):
                            nc.tensor.matmul(
                                lps[:], ones_b[:], pT[:, kc, :],
                                start=(kc == 0), stop=(kc == nch - 1))
                            nc.tensor.matmul(
                                ops[:], v_rb[b][:, kc, :], pT[:, kc, :],
                                start=(kc == 0), stop=(kc == nch - 1))
                        rec = p2w.tile([P, 512], F32, name="rec", tag="rec")
                        nc.vector.reciprocal(rec[:], lps[:])
                        nc.vector.tensor_mul(
                            att_sb[b * QB + qb][:, h, :], ops[:], rec[:])

                    for b in range(B):
                        for kc in range(SCH):
                            vin = qkv_sb[b * QB + kc // 4][
                                :, NQ + 1, (kc % 4) * P:(kc % 4 + 1) * P]
                            tps = ps2.tile([P, P], BF, name="vt", tag="vt")
                            nc.tensor.transpose(tps[:], vin, ident_b[:])
                            nc.vector.tensor_copy(v_rb[b][:, kc, :], tps[:])
                        for qb in range(QB):
                            prev = None
                            for h in range(NQ):
                                pT = p2pt.tile([P, 4 * QB, 512], BF,
                                               name="pT", tag="pT")
                                attn_scores(b, qb, h, pT)
                                if prev is not None:
                                    attn_av(b, qb, *prev)
                                prev = (h, pT)
                            attn_av(b, qb, *prev)

                # ------------- Phase 3: out projection -------------
                if 3 in phases:
                  with tc.tile_pool(name="p3w", bufs=1) as p3w, \
                     tc.tile_pool(name="p3o", bufs=2) as p3o, \
                     tc.tile_pool(name="ps3", bufs=4, space="PSUM") as ps3:
                    wout_t = p3w.tile([P, NQ, H], BF)
                    for g in range(2):
                        nc.sync.dma_start(
                            wout_t[:, :, g * 2048:(g + 1) * 2048],
                            wout_v[:, :, g * 2048:(g + 1) * 2048])
                    for sb in range(NSB):
                        for hg in range(HCH // 8):
                            ost = p3o.tile([P, 8, 512], F16, name="ost",
                                           tag="ost")
                            for hi in range(8):
                                ht = hg * 8 + hi
                                ops3 = ps3.tile([P, 512], F32, name="o3",
                                                tag="o3")
                                for ci in range(NQ):
                                    nc.tensor.matmul(
                                        ops3[:],
                                        wout_t[:, ci, ht * P:(ht + 1) * P],
                                        att_sb[sb][:, ci, :],
                                        start=(ci == 0), stop=(ci == NQ - 1))
                                nc.vector.tensor_copy(ost[:, hi, :], ops3[:])
                            nc.sync.dma_start(
                                outT_v[:, hg * 8:(hg + 1) * 8,
                                       sb * 512:(sb + 1) * 512],
                                ost[:])

        if reps == 1:
            body()
        else:
            with tc.For_i(0, reps, 1) as iv:
                body(iv)
    return nc


def host_inputs(x, cos, sin, Wqkv, Wout, cfg=CFG):
    """Build the 8 per-core input maps from the full-problem inputs."""
    import ml_dtypes
    bf16 = ml_dtypes.bfloat16
    B, S, H, HD, NQ = cfg["B"], cfg["S"], cfg["H"], cfg["HD"], cfg["NQ"]
    SQ = B * S
    NH = NQ * N_CORES          # total q heads
    scale = 1.0 / np.sqrt(HD)

    x = np.asarray(x, dtype=np.float32)
    cos = np.asarray(cos, dtype=np.float32)
    sin = np.asarray(sin, dtype=np.float32)
    Wqkv = np.asarray(Wqkv, dtype=np.float32)
    Wout = np.asarray(Wout, dtype=np.float32)

    xT_b = np.ascontiguousarray(x.reshape(SQ, H).T).astype(bf16)
    cosT2 = np.ascontiguousarray(np.tile(cos.T, (1, B))).astype(bf16)
    sinT = sin.T
    sinS2 = np.concatenate([-sinT[:HD // 2], sinT[HD // 2:]], axis=0)
    sinS2 = np.ascontiguousarray(np.tile(sinS2, (1, B))).astype(bf16)
    qv = np.arange(512)
    pv = np.arange(P)
    mask = np.zeros((P, 4, 512), np.float32)
    for v in range(4):
        mask[:, v, :] = (qv[None, :] >= (v * P + pv)[:, None])
    mask = mask.reshape(P, 4 * 512).astype(bf16)

    in_maps = []
    for c in range(N_CORES):
        wq = Wqkv[:, c * NQ * HD:(c + 1) * NQ * HD] * scale
        wk = Wqkv[:, NH * HD + c * HD: NH * HD + (c + 1) * HD]
        wv = Wqkv[:, NH * HD + N_CORES * HD + c * HD:
                  NH * HD + N_CORES * HD + (c + 1) * HD]
        w3 = np.concatenate([wq, wk, wv], axis=1).astype(bf16)
        wout = Wout[c * NQ * HD:(c + 1) * NQ * HD, :].astype(bf16)
        in_maps.append({
            "xT": xT_b, "w3": w3, "wout": wout,
            "cosT": cosT2, "sinS": sinS2, "maskT": mask,
        })
    return in_maps


class _Runner:
    """Compiled-kernel runner over the axon PJRT path (kept for re-invocation)."""

    def __init__(self, nc, n_cores):
        import jax
        from jax.sharding import Mesh, PartitionSpec
        from jax.experimental.shard_map import shard_map
        from concourse.bass2jax import (
            _bass_exec_p, partition_id_tensor, install_neuronx_cc_hook)
        install_neuronx_cc_hook()
        self.jax = jax
        self.n_cores = n_cores
        partition_name = nc.partition_id_tensor.name if nc.partition_id_tensor else None
        in_names, out_names, out_avals, zero_outs = [], [], [], []
        for alloc in nc.m.functions[0].allocations:
            if not isinstance(alloc, mybir.MemoryLocationSet):
                continue
            name = alloc.memorylocations[0].name
            if alloc.kind == "ExternalInput":
                if name != partition_name:
                    in_names.append(name)
            elif alloc.kind == "ExternalOutput":
                shape = tuple(alloc.tensor_shape)
                dtype = mybir.dt.np(alloc.dtype)
                out_avals.append(jax.core.ShapedArray(shape, dtype))
                out_names.append(name)
                zero_outs.append(np.zeros(shape, dtype))
        self.in_names = in_names[:]
        self.out_names, self.out_avals, self.zero_outs = out_names, out_avals, zero_outs
        self.n_params = len(in_names)
        all_names = in_names + out_names
        if partition_name is not None:
            all_names.append(partition_name)

        def _body(*args):
            operands = list(args)
            if partition_name is not None:
                operands.append(partition_id_tensor())
            outs = _bass_exec_p.bind(
                *operands, out_avals=tuple(out_avals), in_names=tuple(all_names),
                out_names=tuple(out_names), lowering_input_output_aliases=(),
                sim_require_finite=True, sim_require_nnan=True, nc=nc)
            return tuple(outs)

        devices = jax.devices()[:n_cores]
        self.mesh = Mesh(np.asarray(devices), ("core",))
        specs_in = (PartitionSpec("core"),) * (self.n_params + len(out_names))
        specs_out = (PartitionSpec("core"),) * len(out_names)
        self.sharded = jax.jit(
            shard_map(_body, mesh=self.mesh, in_specs=specs_in,
                      out_specs=specs_out, check_rep=False),
            keep_unused=True)
        self._dev_args = None

    def stage(self, in_maps):
        import jax
        from jax.sharding import PartitionSpec
        per_core = [[np.asarray(m[n]) for n in self.in_names] for m in in_maps]
        concat = [np.concatenate([per_core[c][i] for c in range(self.n_cores)], axis=0)
                  for i in range(self.n_params)]
        concat += [np.zeros((self.n_cores * z.shape[0], *z.shape[1:]), z.dtype)
                   for z in self.zero_outs]
        sh = jax.sharding.NamedSharding(self.mesh, PartitionSpec("core"))
        self._dev_args = [jax.device_put(a, sh) for a in concat]
        jax.block_until_ready(self._dev_args)

    def execute(self):
        out = self.sharded(*self._dev_args)
        self.jax.block_until_ready(out)
        return out

    def results(self, out):
        return [
            {n: np.asarray(out[i]).reshape(self.n_cores, *self.out_avals[i].shape)[c]
             for i, n in enumerate(self.out_names)}
            for c in range(self.n_cores)
        ]


_cached = {}


def _get_runner(reps=1):
    key = reps
    if key not in _cached:
        nc = build(CFG, reps=reps)
        nc.compile()
        _cached[key] = _Runner(nc, N_CORES)
    return _cached[key]


def kernel(x, cos, sin, Wqkv, Wout):
    cfg = CFG
    B, S, H = cfg["B"], cfg["S"], cfg["H"]
    runner = _get_runner(reps=1)
    in_maps = host_inputs(x, cos, sin, Wqkv, Wout, cfg)
    runner.stage(in_maps)
    out = runner.execute()
    results = runner.results(out)
    acc = np.zeros((B * S, H), np.float32)
    for c in range(N_CORES):
        acc += results[c]["outT"].T.astype(np.float32)
    return acc.reshape(B, S, H).astype(np.float32)
